# revision 14
# baseline (speedup 1.0000x reference)
"""Trainium2 Bass kernel for a 12-head MHA layer with relative position bias
and a 0/1 attention mask (B=2, N=2048, C=768, H=12, d=64), sharded over 8
NeuronCores (batch x head-group parallel: core c handles batch c//4 and heads
3*(c%4) .. 3*(c%4)+2).

v4: quarter-pipelined schedule. Phase D runs paired ST steps as in v3, but in
block order [DI q0][DI q1][DII q0][DII q1][DI q2][DII q2][DI q3][DII q3] so a
quarter's softmax denominators finish early; the normalization (E) and the
output projection chunk F(q) are interleaved into the later D blocks. Only
E(q3)+F(q3) remain as a short tail, so the PE clock never cools.

Softmax bands (8-step blocks): even blocks take the ACT path (Scalar exp ->
GpSimd stt multiply by an int8 fixed-point eb = round(116*exp(bias))*mask;
the 116 descales via the stt scalar), odd blocks take the fused DVE
Schraudolph path (int16 T = round(A16*bias + B16-C16), mask as -25000
sentinel). eb streams as 8-step mega tiles (int8 [128,8192] / int16) - one
DMA descriptor per mega, cutting trigger cost and raising DMA line size to
8/16KB per partition.

The denominator reciprocal uses vector.reciprocal_approx_fast (no act-table
swaps); the reciprocal row broadcasts via a 1-partition f32r matmul at
tile_position (32q, 0), and osum is normalized in place by a DVE stt.
"""

import os
import numpy as np
import ml_dtypes

import concourse.bass as bass
import concourse.tile as tile
from concourse import bacc, mybir
from concourse.alu_op_type import AluOpType
from concourse.bass_utils import run_bass_kernel_spmd

AF = mybir.ActivationFunctionType
DT = mybir.dt
F32R = mybir.dt.float32r

B, N, C, H, D = 2, 2048, 768, 12, 64
HPC = H // 4          # heads per core (8 cores = 2 batches x 4 head-groups)
NCORES = 8
SCALE = float(D) ** -0.5
NT = N // 128

# ---- fused integer-exp (Schraudolph, int16/bf16-bitcast form) ----
A16 = (1 << 7) / np.log(2.0)          # 184.66496...
B16 = 127 << 7                        # 16256
C16 = 5.1                             # spline-center correction (HW-tuned)
T_MASKED = -25000                     # int16 sentinel -> bf16 ~ -5e-9
EB_SCALE = 116.0                      # int8 eb fixed-point scale

QTR = 512                             # q-quarter width
NQTR = N // QTR
MEGA = 8                              # steps per eb mega tile
NMEGA = 12                            # 96 steps / 8

LAST_RESULTS = None   # BassKernelResults of the most recent kernel() call


def build_schedule():
    """96 paired steps in block order DIq0 DIq1 DIIq0 DIIq1 DIq2 DIIq2 DIq3
    DIIq3. Fused (Schraudolph) = odd 8-step blocks; ACT (int8 eb) = even."""
    steps = []
    for qtr in (0, 1):
        for j in range(NT):
            steps.append(((0, j, qtr), (1, j, qtr)))
    for qtr in (0, 1):
        for jj in range(NT // 2):
            steps.append(((2, 2 * jj, qtr), (2, 2 * jj + 1, qtr)))
    for j in range(NT):
        steps.append(((0, j, 2), (1, j, 2)))
    for jj in range(NT // 2):
        steps.append(((2, 2 * jj, 2), (2, 2 * jj + 1, 2)))
    for j in range(NT):
        steps.append(((0, j, 3), (1, j, 3)))
    for jj in range(NT // 2):
        steps.append(((2, 2 * jj, 3), (2, 2 * jj + 1, 3)))
    assert len(steps) == 96
    # fused (Schraudolph) = odd step-pairs, ACT (int8 eb) = even pairs so the
    # Scalar and DVE softmax lanes drain concurrently
    sch_steps = tuple(s for s in range(len(steps)) if (s // 2) % 2 == 1)
    return steps, sch_steps


def build_program(n=N, c_in=C, hpc=HPC, d=D, c_out=C):
    nt = n // 128
    qch = _q_chunks(n)
    ck = (c_in + 127) // 128
    n_qk_chunks = 2 * ((hpc + 1) // 2)  # 4 for hpc=3
    wqk_cols = 128 * n_qk_chunks
    wv_cols = hpc * (d + 2)
    mo = c_out // 128

    steps, sch_steps = build_schedule()

    def pc(kc):
        return min(128, c_in - 128 * kc)

    nc = bacc.Bacc("TRN2", target_bir_lowering=False, debug=False)
    xt = nc.dram_tensor("xt", [c_in, n], DT.bfloat16, kind="ExternalInput").ap()
    wqk = nc.dram_tensor("wqk", [c_in, wqk_cols], DT.bfloat16, kind="ExternalInput").ap()
    wv = nc.dram_tensor("wv", [c_in, wv_cols], DT.bfloat16, kind="ExternalInput").ap()
    eba = nc.dram_tensor("eba", [NMEGA // 2, 128, MEGA * 2 * QTR], DT.int8,
                         kind="ExternalInput").ap()
    ebi = nc.dram_tensor("ebi", [NMEGA // 2, 128, MEGA * 2 * QTR], DT.int16,
                         kind="ExternalInput").ap()
    pw = nc.dram_tensor("pw", [hpc * d, c_out], DT.bfloat16, kind="ExternalInput").ap()
    yt = nc.dram_tensor("yt", [c_out, n], DT.bfloat16, kind="ExternalOutput").ap()

    with tile.TileContext(nc) as tc:
        # ---- pools ----
        persist = tc.alloc_tile_pool(name="persist", bufs=1)
        qkvout = tc.alloc_tile_pool(name="qkvout", bufs=1)
        loadp = tc.alloc_tile_pool(name="loadp", bufs=1)
        ebp8 = tc.alloc_tile_pool(name="ebp8", bufs=3)
        ebpI = tc.alloc_tile_pool(name="ebpI", bufs=3)
        ps_qkv = tc.alloc_tile_pool(name="ps_qkv", bufs=4, space="PSUM")

        pwA_s = persist.tile([128, c_out], DT.bfloat16, tag="pwA")
        pwB_s = persist.tile([64, c_out], DT.bfloat16, tag="pwB")
        ones3 = persist.tile([128, hpc], DT.float32, tag="ones3")
        nc.vector.memset(ones3, 1.0)

        qk_s = qkvout.tile([128, n_qk_chunks, n], DT.bfloat16, tag="qk")
        v_s = qkvout.tile([128, nt, wv_cols], DT.bfloat16, tag="v")

        xts = loadp.tile([128, ck, n], DT.bfloat16, tag="xts")
        wqk_s = loadp.tile([128, ck, wqk_cols], DT.bfloat16, tag="wqk")
        wv_s = loadp.tile([128, ck, wv_cols], DT.bfloat16, tag="wv")

        # input loads: xt/wqk first (prologue compute), eb megas interleaved.
        # A mega holds one 16-step window's worth of its parity's pairs
        # (4 pairs x 2048 cols).
        mega_tiles = {}

        def emit_mega(kind, w):
            if kind == "a":
                t = ebp8.tile([128, MEGA * 2 * QTR], DT.int8, tag="eb8",
                              name=f"eb8_{w}")
                nc.sync.dma_start(out=t, in_=eba[w])
            else:
                t = ebpI.tile([128, MEGA * 2 * QTR], DT.int16, tag="ebI",
                              name=f"ebI_{w}")
                nc.sync.dma_start(out=t, in_=ebi[w])
            mega_tiles[(kind, w)] = t

        for kc in range(ck):
            p = pc(kc)
            nc.gpsimd.dma_start(out=xts[:p, kc, :], in_=xt[128 * kc:128 * kc + p, :])
            nc.gpsimd.dma_start(out=wqk_s[:p, kc, :], in_=wqk[128 * kc:128 * kc + p, :])
        emit_mega("a", 0)
        emit_mega("i", 0)
        for kc in range(ck):
            p = pc(kc)
            nc.gpsimd.dma_start(out=wv_s[:p, kc, :], in_=wv[128 * kc:128 * kc + p, :])
        nc.gpsimd.dma_start(out=pwA_s, in_=pw[0:128, :])
        nc.gpsimd.dma_start(out=pwB_s, in_=pw[128:hpc * 64, :])
        emit_mega("a", 1)
        emit_mega("i", 1)

        # ---- phase B/C emitters ----
        def emit_qk_group(m, fo, fs, pool, tag, width):
            ps = pool.tile([128, width], DT.float32, tag=tag, name=f"psB{m}")
            for kc in range(ck):
                p = pc(kc)
                nc.tensor.matmul(
                    ps[:, :fs],
                    lhsT=wqk_s[:p, kc, 128 * m:128 * m + 128],
                    rhs=xts[:p, kc, fo:fo + fs],
                    start=(kc == 0), stop=(kc == ck - 1),
                )
            nc.scalar.copy(qk_s[:, m, fo:fo + fs], ps[:, :fs])

        def emit_v_group(j, pool, tag, width):
            ps = pool.tile([128, width], DT.float32, tag=tag, name=f"psC{j}")
            psv = ps[:, :wv_cols]
            for kc in range(ck):
                p = pc(kc)
                nc.tensor.matmul(
                    psv,
                    lhsT=xts[:p, kc, 128 * j:128 * j + 128],
                    rhs=wv_s[:p, kc, :],
                    start=(kc == 0), stop=(kc == ck - 1),
                )
            nc.scalar.copy(v_s[:, j, :], psv)
            nc.vector.tensor_copy(
                v_s[:, j, :].rearrange("p (h c) -> p h c", c=d + 2)[:, :, d],
                ones3)

        # prologue: h0/h1 q+k projections and the first two v' tiles
        for m in (0, 1):
            for (fo, fs) in qch:
                emit_qk_group(m, fo, fs, ps_qkv, "psqkv", 512)
        for j in (0, 1):
            emit_v_group(j, ps_qkv, "psqkv", 512)

        ps_qkv.release()

        # ---- attention pools ----
        e1pool = tc.alloc_tile_pool(name="e1pool", bufs=4)
        normp = tc.alloc_tile_pool(name="normp", bufs=1)
        ytp = tc.alloc_tile_pool(name="ytp", bufs=2)
        ps_st = tc.alloc_tile_pool(name="ps_st", bufs=2, space="PSUM")
        ps_ot = tc.alloc_tile_pool(name="ps_ot", bufs=3, space="PSUM")
        ps_pj = tc.alloc_tile_pool(name="ps_pj", bufs=1, space="PSUM")

        osum_all = normp.tile([66, hpc, n], DT.bfloat16, tag="osum")
        osum = [osum_all[:, i, :] for i in range(hpc)]
        osum01 = normp.tile([128, n], DT.bfloat16, tag="osum01")
        den4 = normp.tile([97, hpc * QTR], DT.bfloat16, tag="den4")
        nc.vector.memset(den4, 1.0)
        denf = normp.tile([97, hpc * QTR], DT.float32, tag="denf")
        rrowf = normp.tile([97, hpc * QTR], DT.float32, tag="rrowf")
        rrowb = normp.tile([97, hpc * QTR], DT.bfloat16, tag="rrowb")
        ones4 = normp.tile([97, 128], DT.bfloat16, tag="ones4")
        nc.vector.memset(ones4, 1.0)

        def unit_aps(head, j):
            if head < 2:
                base = 64 * head
                qv = qk_s[base:base + 64, 2 * (head // 2), :]
                kv = qk_s[base:base + 64, 2 * (head // 2) + 1, :]
            else:
                base = 64 * (j % 2)
                qv = qk_s[base:base + 64, 2, :]
                kv = qk_s[base:base + 64, 3, :]
            return qv, kv

        def emit_recip(q, seg0, nseg):
            r = slice(32 * q, 32 * q + 1)
            cs = slice(seg0 * QTR, (seg0 + nseg) * QTR)
            nc.scalar.copy(denf[r, cs], den4[r, cs])
            nc.vector.reciprocal(rrowf[r, cs], denf[r, cs])
            nc.vector.tensor_copy(rrowb[r, cs], rrowf[r, cs])

        def emit_rps_norm(i, q):
            rps = ps_pj.tile([64, QTR], DT.float32, tag="pj", name=f"rps{i}_{q}")
            nc.tensor.matmul(
                rps,
                lhsT=ones4[32 * q:32 * q + 1, 0:64],
                rhs=rrowb[32 * q:32 * q + 1, i * QTR:i * QTR + QTR],
                start=True, stop=True, tile_position=(32 * q, 0),
            )
            qo = q * QTR
            nc.vector.scalar_tensor_tensor(
                osum[i][0:64, qo:qo + QTR], rps, 1.0,
                osum[i][0:64, qo:qo + QTR],
                AluOpType.mult, AluOpType.mult)

        def emit_repack(q):
            qo = q * QTR
            nc.gpsimd.dma_start(out=osum01[0:64, qo:qo + QTR],
                                in_=osum[0][0:64, qo:qo + QTR])
            nc.gpsimd.dma_start(out=osum01[64:128, qo:qo + QTR],
                                in_=osum[1][0:64, qo:qo + QTR])

        def emit_f(q, m):
            ps = ps_pj.tile([128, QTR], DT.float32, tag="pj", name=f"pj{q}_{m}")
            qo = q * QTR
            nc.tensor.matmul(
                ps, lhsT=pwA_s[:, 128 * m:128 * m + 128],
                rhs=osum01[:, qo:qo + QTR], start=True, stop=False)
            nc.tensor.matmul(
                ps, lhsT=pwB_s[:, 128 * m:128 * m + 128],
                rhs=osum[2][0:64, qo:qo + QTR], start=False, stop=True)
            yts = ytp.tile([128, QTR], DT.bfloat16, tag="yts")
            nc.scalar.copy(yts, ps)
            nc.sync.dma_start(out=yt[128 * m:128 * m + 128, qo:qo + QTR], in_=yts)

        # interleave plan
        inter = {}

        def add(s, task):
            inter.setdefault(s, []).append(task)

        for j in range(2, nt):
            add(j - 2, ("C", j))
        bgroups = [(m, fo, fs) for m in (2, 3) for (fo, fs) in qch]
        for g, bg in enumerate(bgroups):
            add(14 + 2 * g, ("B", bg))
        for w in range(2, 6):
            add(16 * w - 24, ("MEGA", "a", w))
            add(16 * w - 22, ("MEGA", "i", w))
        # E/F per quarter (den ready after steps: q0:39 q1:47 q2:71 q3:95)
        for q, (s_re, s_rp, s_fp, df) in enumerate((
                (41, 42, 48, 2), (49, 51, 60, 2),
                (73, 74, 78, 1), (89, 90, None, None))):
            add(s_re, ("RECIP", q, 0, 3 if q < 3 else 2))
            for i in range(3 if q < 3 else 2):
                add(s_rp + (2 * i if q == 1 else i), ("RPSN", i, q))
            add(s_rp + (6 if q == 1 else 3), ("REPACK", q))
            if s_fp is not None:
                for m in range(mo):
                    add(s_fp + df * m, ("F", q, m))

        def run_task(task):
            kind = task[0]
            if kind == "C":
                emit_v_group(task[1], ps_pj, "pj", 512)
            elif kind == "B":
                m, fo, fs = task[1]
                emit_qk_group(m, fo, fs, ps_pj, "pj", 512)
            elif kind == "MEGA":
                emit_mega(task[1], task[2])
            elif kind == "RECIP":
                emit_recip(task[1], task[2], task[3])
            elif kind == "RPSN":
                emit_rps_norm(task[1], task[2])
            elif kind == "REPACK":
                emit_repack(task[1])
            elif kind == "F":
                emit_f(task[1], task[2])

        # ---- phase D ----
        ots = {}
        for s, (ua, ub) in enumerate(steps):
            for task in inter.get(s, ()):
                run_task(task)

            w = s // 16
            ebt = mega_tiles[("i" if s in sch_steps else "a", w)]
            ecol = ((s % 16) // 4) * 4 * QTR + (s % 2) * 2 * QTR

            st = ps_st.tile([128, 2 * QTR], DT.float32, tag="st")
            for half, (h, j, qtr) in enumerate((ua, ub)):
                qv, kv = unit_aps(h, j)
                nc.tensor.matmul(
                    st[:, half * QTR:(half + 1) * QTR],
                    lhsT=kv[:, 128 * j:128 * j + 128],
                    rhs=qv[:, qtr * QTR:qtr * QTR + QTR],
                    start=True, stop=True,
                )

            if s in sch_steps:
                e1i = e1pool.tile([128, 2 * QTR], DT.int16, tag="e1i",
                                  name="e1i")
                nc.vector.scalar_tensor_tensor(
                    e1i, st, float(A16),
                    ebt[:, ecol:ecol + 2 * QTR],
                    AluOpType.mult, AluOpType.add)
                e1x = e1i.bitcast(DT.bfloat16)
            else:
                e0 = e1pool.tile([128, 2 * QTR], DT.bfloat16, tag="e0",
                                 name="e0")
                nc.scalar.activation(e0, st, AF.Exp)
                e1 = e1pool.tile([128, 2 * QTR], DT.bfloat16, tag="e1",
                                 name="e1")
                # e1 = e0 * eb8 (x116-scaled; fused path carries the same
                # scale via its T offset, and it cancels in normalization)
                nc.gpsimd.tensor_tensor(
                    e1, e0, ebt[:, ecol:ecol + 2 * QTR], AluOpType.mult)
                e1x = e1

            for half, (h, j, qtr) in enumerate((ua, ub)):
                key = (h, qtr)
                if key not in ots:
                    ots[key] = (ps_ot.tile([66, QTR], DT.float32, tag="ot",
                                           name=f"ot{h}_{qtr}"), [0])
                ot, cnt = ots[key]
                nc.tensor.matmul(
                    ot,
                    lhsT=v_s[:, j, (d + 2) * h:(d + 2) * h + d + 2],
                    rhs=e1x[:, half * QTR:(half + 1) * QTR],
                    start=(cnt[0] == 0), stop=(cnt[0] == nt - 1),
                )
                cnt[0] += 1
                if cnt[0] == nt:
                    nc.vector.tensor_copy(
                        osum[h][:, qtr * QTR:qtr * QTR + QTR], ot)
                    nc.sync.dma_start(
                        out=den4[32 * qtr:32 * qtr + 1,
                                 h * QTR:h * QTR + QTR],
                        in_=osum[h][64:65, qtr * QTR:qtr * QTR + QTR])
                    del ots[key]

        # ---- tail: E(q3) h2 + F(q3) ----
        emit_recip(3, 2, 1)
        emit_rps_norm(2, 3)
        for m in range(mo):
            emit_f(3, m)

        ps_pj.release()
        ps_ot.release()
        ps_st.release()
        ytp.release()
        normp.release()
        e1pool.release()
        ebpI.release()
        ebp8.release()
        loadp.release()
        qkvout.release()
        persist.release()

    nc.compile()
    return nc


def _q_chunks(n, c=512):
    out = []
    o = 0
    while o < n:
        sz = min(c, n - o)
        out.append((o, sz))
        o += sz
    return out


_PROG = {}


def _get_program(**kw):
    key = tuple(sorted(kw.items()))
    if key not in _PROG:
        _PROG[key] = build_program(**kw)
    return _PROG[key]


def make_in_maps(x, mask, qkv_w, qkv_b, rel_bias, proj_w):
    x = np.asarray(x, dtype=np.float32)
    mask = np.asarray(mask)
    qkv_w = np.asarray(qkv_w, dtype=np.float32)
    qkv_b = np.asarray(qkv_b, dtype=np.float32)
    rel_bias = np.asarray(rel_bias, dtype=np.float32)
    proj_w = np.asarray(proj_w, dtype=np.float32)

    n_qk_chunks = 2 * ((HPC + 1) // 2)
    wqk_cols = 128 * n_qk_chunks
    wv_cols = HPC * (D + 2)
    has_bias = bool(np.any(qkv_b))
    c_in = C + 1 if has_bias else C

    steps, sch_steps = build_schedule()

    xts = []
    for b in range(B):
        xb = x[b].T
        if has_bias:
            xb = np.concatenate([xb, np.ones((1, N), np.float32)], axis=0)
        xts.append(np.ascontiguousarray(xb))

    maps = []
    for core in range(NCORES):
        b = core // 4
        heads = [HPC * (core % 4) + i for i in range(HPC)]

        wqk = np.zeros((c_in, wqk_cols), np.float32)
        wv = np.zeros((c_in, wv_cols), np.float32)
        pwm = np.zeros((HPC * D, C), np.float32)
        for i, h in enumerate(heads):
            base = 128 * (2 * (i // 2)) + 64 * (i % 2)
            wqk[:C, base:base + 64] = qkv_w[D * h:D * h + D, :].T * SCALE
            kbase = 128 * (2 * (i // 2) + 1) + 64 * (i % 2)
            wqk[:C, kbase:kbase + 64] = qkv_w[C + D * h:C + D * h + D, :].T
            wv[:C, (D + 2) * i:(D + 2) * i + D] = qkv_w[2 * C + D * h:2 * C + D * h + D, :].T
            if has_bias:
                wqk[C, base:base + 64] = qkv_b[D * h:D * h + D] * SCALE
                wqk[C, kbase:kbase + 64] = qkv_b[C + D * h:C + D * h + D]
                wv[C, (D + 2) * i:(D + 2) * i + D] = qkv_b[2 * C + D * h:2 * C + D * h + D]
            pwm[64 * i:64 * i + 64, :] = proj_w[:, D * h:D * h + D].T
        if HPC % 2 == 1:
            i = HPC - 1
            base = 128 * (2 * (i // 2))
            kbase = base + 128
            wqk[:, base + 64:base + 128] = wqk[:, base:base + 64]
            wqk[:, kbase + 64:kbase + 128] = wqk[:, kbase:kbase + 64]

        mb = (mask[b, 0] != 0)
        bTs = [rel_bias[h].T for h in heads]
        mT = mb.T
        eba = np.zeros((NMEGA // 2, 128, MEGA * 2 * QTR), np.int8)
        ebI = np.zeros((NMEGA // 2, 128, MEGA * 2 * QTR), np.int16)
        for s, (ua, ub) in enumerate(steps):
            w = s // 16
            fused = s in sch_steps
            base = ((s % 16) // 4) * 4 * QTR + (s % 2) * 2 * QTR
            for k, (h, j, qtr) in enumerate((ua, ub)):
                rows = slice(128 * j, 128 * j + 128)
                cols = slice(qtr * QTR, qtr * QTR + QTR)
                bT = bTs[h][rows, cols]
                mTk = mT[rows, cols]
                dst = slice(base + k * QTR, base + (k + 1) * QTR)
                if fused:
                    # + A16*ln(EB_SCALE): matches the ACT path's x116 scale
                    t = np.rint(A16 * bT + (B16 - C16)
                                + A16 * np.log(EB_SCALE))
                    ebI[w, :, dst] = np.where(
                        mTk, t, float(T_MASKED)).astype(np.int16)
                else:
                    eba[w, :, dst] = np.clip(
                        np.rint(np.exp(bT) * mTk * EB_SCALE),
                        -128, 127).astype(np.int8)

        maps.append({
            "xt": xts[b].astype(ml_dtypes.bfloat16),
            "wqk": wqk.astype(ml_dtypes.bfloat16),
            "wv": wv.astype(ml_dtypes.bfloat16),
            "eba": eba,
            "ebi": ebI,
            "pw": pwm.astype(ml_dtypes.bfloat16),
        })
    return maps, has_bias


def kernel(x, mask, qkv_w, qkv_b, rel_bias, proj_w, proj_b):
    global LAST_RESULTS
    maps, has_bias = make_in_maps(x, mask, qkv_w, qkv_b, rel_bias, proj_w)
    nc = _get_program(c_in=C + 1 if has_bias else C)

    trace = bool(os.environ.get("KERNEL_TRACE"))
    try:
        res = run_bass_kernel_spmd(
            nc, maps, list(range(NCORES)),
            trace=trace,
            trace_cores=list(range(NCORES)) if trace else None,
        )
    except Exception:
        if not trace:
            raise
        os.environ["BASS_NEVER_TRACE"] = "1"
        res = run_bass_kernel_spmd(nc, maps, list(range(NCORES)), trace=False)
    LAST_RESULTS = res

    proj_b = np.asarray(proj_b, dtype=np.float32)
    out = np.empty((B, N, C), np.float32)
    for b in range(B):
        acc = res.results[4 * b]["yt"].astype(np.float32)
        for c in range(4 * b + 1, 4 * b + 4):
            acc = acc + res.results[c]["yt"]
        out[b] = acc.T + proj_b[None, :]
    return out
